# revision 29
# baseline (speedup 1.0000x reference)
"""DCGRU cell on 8 Trainium2 NeuronCores (data-parallel over batch).

Design (v1, feature-major + fp8 DoubleRow):
  - All diffusion terms are direct functions of x0: with the Chebyshev
    fold x2 = 2*S^2 x0 - x0, host precomputes S^T and (S^2)^T per
    support and folds the constants into the projection weights
    (W0' = W0 - W2 - W4, W2' = 2 W2, W4' = 2 W4).  No chained spmm.
  - spmm runs feature-major: stationary = x0 natural node-major blocks,
    moving = S^T column chunks.  Output y^T = (S x)^T lands directly in
    the (batch,feature)-partition layout the projection consumes, so no
    PE transposes of diffusion outputs are needed.
  - Diffusion matmuls are fp8e4m3 with DoubleRow perf mode (256-node
    contraction per instruction, 0.5 cycles/row).  Each S matrix is
    pre-scaled by a power of two into fp8's normal range (S^2 is
    otherwise entirely subnormal); the inverse scale is folded into the
    bf16 projection weights.  Simulated end-to-end rel err: 3.4e-3.
  - Projection stays bf16: stationaries are 2-batch block-diagonal W
    tiles; the (m, input-feature) terms contract via an 80-partition
    packed xin^T tile shared by both dconvs.
  - Gate outputs stay feature-major: u^T kept in SBUF, candidate input
    candX^T = sigmoid(r)^T * state^T built feature-major; 64 small
    transposes produce the fp8 node-major candX stationary.  The final
    GRU combine runs feature-major and the host un-transposes.
"""

import numpy as np

import concourse.bass as bass
from concourse import bacc
import concourse.mybir as mybir
import concourse.tile as tile
from concourse.bass_utils import run_bass_kernel_spmd
from concourse.masks import make_identity

N = 2048            # nodes
B = 64              # global batch
BL = 8              # batch per core
NCORES = 8
D_IN = 2
U = 64              # hidden units
M = 5               # 1 + 2 supports * 2 steps
F = D_IN + U        # 66
NB = N // 128       # 16 node blocks
SC = BL * U         # 512 state cols in natural layout
IC = BL * D_IN      # 16 input cols
CW = SC + IC        # 528 natural cols per node block
NCH = 512           # node chunk (psum free size)
NC4 = N // NCH      # 4 chunks
J = BL // 2         # 4 batch pairs
PKM = 16            # packed input rows per m (8 b * 2 fi)

F32 = mybir.dt.float32
BF16 = mybir.dt.bfloat16
FP8 = mybir.dt.float8e4
DR = mybir.MatmulPerfMode.DoubleRow


def _build_nc():
    nc = bacc.Bacc(None, target_bir_lowering=False)

    x0d = nc.declare_dram_parameter("x0", [N, CW], FP8, isOutput=False)
    stTd = nc.declare_dram_parameter("stT", [128, J * N], BF16, isOutput=False)
    xind = nc.declare_dram_parameter("xin", [5 * PKM, N], BF16, isOutput=False)
    sd = [nc.declare_dram_parameter(f"s{m}", [N, N], FP8, isOutput=False)
          for m in range(1, 5)]
    wgsd = nc.declare_dram_parameter("wgs", [128, 10 * 128], BF16, isOutput=False)
    wcsd = nc.declare_dram_parameter("wcs", [128, 5 * 128], BF16, isOutput=False)
    wigd = nc.declare_dram_parameter("wig", [5 * PKM, 8 * 128], BF16, isOutput=False)
    wicd = nc.declare_dram_parameter("wic", [5 * PKM, 4 * 128], BF16, isOutput=False)
    bgd = nc.declare_dram_parameter("bg", [128, 2], F32, isOutput=False)
    bcd = nc.declare_dram_parameter("bc", [128, 1], F32, isOutput=False)
    outd = nc.declare_dram_parameter("out", [128, J * N], BF16, isOutput=True)

    with tile.TileContext(nc) as tc:
        _emit(nc, tc, x0d, stTd, xind, sd, wgsd, wcsd, wigd, wicd,
              bgd, bcd, outd)
    nc.compile()
    return nc


def _emit(nc, tc, x0d, stTd, xind, sd, wgsd, wcsd, wigd, wicd,
          bgd, bcd, outd):
    from contextlib import ExitStack
    ctx = ExitStack()
    with ctx:
        consts = ctx.enter_context(tc.tile_pool(name="consts", bufs=1))
        acts = ctx.enter_context(tc.tile_pool(name="acts", bufs=1))
        spool = ctx.enter_context(tc.tile_pool(name="spool", bufs=2))
        small = ctx.enter_context(tc.tile_pool(name="small", bufs=3))
        psum = ctx.enter_context(tc.tile_pool(name="psum", bufs=8, space="PSUM"))

        # x0 load goes first so diffusion can start immediately; everything
        # else trails it in the DMA queues.
        x0n = acts.tile([128, NB * CW], FP8, tag="x0n")
        x0dv = x0d.rearrange("(t p) c -> p t c", p=128)
        nc.scalar.dma_start(x0n[:].rearrange("p (t c) -> p t c", c=CW), x0dv)

        ident = consts.tile([128, 128], F32)
        make_identity(nc, ident[:])
        identb = consts.tile([128, 128], BF16)
        nc.vector.tensor_copy(identb[:], ident[:])

        wgs = consts.tile([128, 10 * 128], BF16)
        wcs = consts.tile([128, 5 * 128], BF16)
        wig = consts.tile([5 * PKM, 8 * 128], BF16)
        wic = consts.tile([5 * PKM, 4 * 128], BF16)
        bg = consts.tile([128, 2], F32)
        bc = consts.tile([128, 1], F32)

        def load_gate_consts():
            for dst, sr in ((wgs, wgsd), (wig, wigd), (bg, bgd)):
                nc.scalar.dma_start(dst[:], sr[:])

        def load_cand_consts():
            for dst, sr in ((wcs, wcsd), (wic, wicd), (bc, bcd)):
                nc.scalar.dma_start(dst[:], sr[:])

        # activations
        cxn = acts.tile([128, NB * CW], FP8, tag="cxn")      # natural candX
        stT = acts.tile([128, J * N], BF16, tag="stT")       # state^T
        uT = acts.tile([128, J * N], BF16, tag="uT")
        cxT = acts.tile([128, J * N], BF16, tag="cxT")       # candX^T (state)
        # packed xin^T [(m, b, fi), n], all five m host-precomputed
        xinT = acts.tile([5 * PKM, N], BF16, tag="xinT")
        xsT = acts.tile([128, 16 * NCH], BF16, tag="xsT")    # (m-1, j) chunk slices
        # resident S^T tiles for m=1 (S_a) and m=3 (S_b): loaded chunkwise
        # during the gate phase, reused without DMA in the candidate phase
        sres = {1: acts.tile([128, NB * N], FP8, tag="s1r", name="s1r"),
                3: acts.tile([128, NB * N], FP8, tag="s3r", name="s3r")}


        sdv = [s.rearrange("(jb p) n -> p jb n", p=128) for s in sd]

        def xsT_s(m, j):
            return xsT[:, ((m - 1) * J + j) * NCH:((m - 1) * J + j + 1) * NCH]

        def s_view(c, m, load):
            """Access view of S_m^T chunk c; DMA it if needed."""
            if m in sres:
                scv = sres[m][:].rearrange(
                    "p (jb n) -> p jb n", n=N)[:, :, c * NCH:(c + 1) * NCH]
                if load:
                    nc.sync.dma_start(
                        scv, sdv[m - 1][:, :, c * NCH:(c + 1) * NCH])
            else:
                sc = spool.tile([128, NB * NCH], FP8, tag="sc",
                                name=f"sc{m}")
                scv = sc[:].rearrange("p (jb n) -> p jb n", n=NCH)
                nc.sync.dma_start(
                    scv, sdv[m - 1][:, :, c * NCH:(c + 1) * NCH])
            return scv

        def diffuse_g(scvs, src, g):
            """One batch-pair group g of chunk work: all four S mats."""
            srcv = src[:].rearrange("p (t w) -> p t w", w=CW)
            c0, c1 = g * 128, (g + 1) * 128
            for m in range(1, 5):
                pt = psum.tile([128, NCH], F32, tag="ps", name=f"pd{m}")
                for t in range(8):
                    nc.tensor.matmul(
                        pt[:],
                        srcv[:, 2 * t:2 * t + 2, c0:c1],
                        scvs[m][:, 2 * t:2 * t + 2, :],
                        start=(t == 0), stop=(t == 7), perf_mode=DR)
                nc.vector.tensor_copy(xsT_s(m, g)[:], pt[:])

        def gate_proj(c, j):
                stTs = stT[:, j * N + c * NCH:j * N + (c + 1) * NCH]
                for h in range(2):
                    pp = psum.tile([128, NCH], F32, tag="ps", name="pproj")
                    nc.tensor.matmul(pp[:], wgs[:, h * 128:(h + 1) * 128],
                                     stTs, start=True, stop=False)
                    for m in range(1, 5):
                        nc.tensor.matmul(
                            pp[:], wgs[:, (2 * m + h) * 128:(2 * m + h + 1) * 128],
                            xsT_s(m, j), start=False, stop=False)
                    nc.tensor.matmul(
                        pp[:], wig[:, (2 * j + h) * 128:(2 * j + h + 1) * 128],
                        xinT[:, c * NCH:(c + 1) * NCH],
                        start=False, stop=True)
                    if h == 0:
                        rT = small.tile([128, NCH], BF16, tag="rT")
                        nc.scalar.activation(
                            rT[:], pp[:],
                            mybir.ActivationFunctionType.Sigmoid,
                            bias=bg[:, 0:1])
                        nc.vector.tensor_mul(
                            cxT[:, j * N + c * NCH:j * N + (c + 1) * NCH],
                            rT[:], stTs)
                    else:
                        nc.scalar.activation(
                            uT[:, j * N + c * NCH:j * N + (c + 1) * NCH], pp[:],
                            mybir.ActivationFunctionType.Sigmoid,
                            bias=bg[:, 1:2])

        def candx_nat(c, j):
                tp = psum.tile([128, NCH], BF16, tag="ps", name="ptr")
                for nb in range(4):
                    nc.tensor.transpose(
                        tp[:, nb * 128:(nb + 1) * 128],
                        cxT[:, j * N + c * NCH + nb * 128:
                            j * N + c * NCH + (nb + 1) * 128],
                        identb[:])
                for nb in range(4):
                    i = c * 4 + nb
                    nc.vector.tensor_copy(
                        cxn[:, i * CW + j * 128:i * CW + (j + 1) * 128],
                        tp[:, nb * 128:(nb + 1) * 128])

        def cand_proj(c, j):
                stTs = stT[:, j * N + c * NCH:j * N + (c + 1) * NCH]
                pp = psum.tile([128, NCH], F32, tag="ps", name="pproj")
                nc.tensor.matmul(pp[:], wcs[:, 0:128],
                                 cxT[:, j * N + c * NCH:j * N + (c + 1) * NCH],
                                 start=True, stop=False)
                for m in range(1, 5):
                    nc.tensor.matmul(pp[:], wcs[:, m * 128:(m + 1) * 128],
                                     xsT_s(m, j), start=False, stop=False)
                nc.tensor.matmul(pp[:], wic[:, j * 128:(j + 1) * 128],
                                 xinT[:, c * NCH:(c + 1) * NCH],
                                 start=False, stop=True)
                cT = small.tile([128, NCH], BF16, tag="cT")
                nc.scalar.activation(cT[:], pp[:],
                                     mybir.ActivationFunctionType.Tanh,
                                     bias=bc[:])
                ot = small.tile([128, NCH], BF16, tag="ot")
                uTs = uT[:, j * N + c * NCH:j * N + (c + 1) * NCH]
                nc.vector.tensor_sub(ot[:], stTs, cT[:])
                nc.vector.tensor_mul(ot[:], ot[:], uTs)
                nc.vector.tensor_add(ot[:], ot[:], cT[:])
                nc.scalar.dma_start(
                    outd[:, j * N + c * NCH:j * N + (c + 1) * NCH], ot[:])

        def diffuse_m(scvs, src, m, tail=None):
            """All four batch-pair groups for S-matrix m (m-major).
            tail(g), if given, emits the group's projection right after
            its diffusion so the chunk's epilogue pipelines."""
            srcv = src[:].rearrange("p (t w) -> p t w", w=CW)
            for g in range(4):
                c0, c1 = g * 128, (g + 1) * 128
                pt = psum.tile([128, NCH], F32, tag="ps", name=f"pd{g}")
                for t in range(8):
                    nc.tensor.matmul(
                        pt[:],
                        srcv[:, 2 * t:2 * t + 2, c0:c1],
                        scvs[m][:, 2 * t:2 * t + 2, :],
                        start=(t == 0), stop=(t == 7), perf_mode=DR)
                nc.vector.tensor_copy(xsT_s(m, g)[:], pt[:])
                if tail is not None:
                    tail(g)

        # ---- gate ----
        for c in range(NC4):
            scvs = {1: s_view(c, 1, True), 2: s_view(c, 2, True)}
            if c == 0:
                nc.gpsimd.dma_start(stT[:], stTd[:])
                load_gate_consts()
                nc.gpsimd.dma_start(xinT[:], xind[:])
            scvs[3] = s_view(c, 3, True)
            scvs[4] = s_view(c, 4, True)
            for m in range(1, 4):
                diffuse_m(scvs, x0n, m)
            diffuse_m(scvs, x0n, 4,
                      tail=lambda g: (gate_proj(c, g), candx_nat(c, g)))
            if c == 0:
                load_cand_consts()
        # ---- candidate ----
        for c in range(NC4):
            scvs = {m: s_view(c, m, False) for m in range(1, 5)}
            for m in range(1, 4):
                diffuse_m(scvs, cxn, m)
            diffuse_m(scvs, cxn, 4, tail=lambda g: cand_proj(c, g))


_NC_CACHE = {}


def _get_nc():
    if "nc" not in _NC_CACHE:
        _NC_CACHE["nc"] = _build_nc()
    return _NC_CACHE["nc"]


def _host_prep(inputs, state, edges1, vals1, edges2, vals2, W_gate, b_gate,
               W_cand, b_cand):
    import ml_dtypes
    BF = ml_dtypes.bfloat16
    # values kept <= 224 so encodings are identical under e4m3 and e4m3fn
    F8 = ml_dtypes.float8_e4m3
    inputs = np.asarray(inputs, np.float32)
    state = np.asarray(state, np.float32)

    def densify_T(edges, vals):
        ST = np.zeros((N, N), np.float32)
        np.add.at(ST, (np.asarray(edges[1]).astype(np.int64),
                       np.asarray(edges[0]).astype(np.int64)),
                  np.asarray(vals, np.float32))
        return ST

    SaT = densify_T(edges1, vals1)
    SbT = densify_T(edges2, vals2)
    Sa2T = SaT @ SaT
    Sb2T = SbT @ SbT
    smats, sscale = [], []
    smatsT = [SaT, Sa2T, SbT, Sb2T]
    for S in smatsT:
        s = 2.0 ** np.floor(np.log2(224.0 / np.abs(S).max()))
        smats.append((S * s).astype(F8))
        sscale.append(s)

    def fold(Wmat):
        Wm = np.asarray(Wmat, np.float32).reshape(F, M, -1).copy()
        Wl = [Wm[:, 0] - Wm[:, 2] - Wm[:, 4], Wm[:, 1], 2.0 * Wm[:, 2],
              Wm[:, 3], 2.0 * Wm[:, 4]]
        Wli = [w[:D_IN].copy() for w in Wl]      # input rows, unscaled
        for m in range(1, 5):
            Wl[m] = Wl[m] / sscale[m - 1]        # state rows absorb 1/s_m
        return Wl, Wli

    def blockdiag2(Wst):
        O = Wst.shape[1]
        Z = np.zeros((128, 2 * O), np.float32)
        Z[:64, :O] = Wst
        Z[64:, O:] = Wst
        return Z

    Wgl, Wgli = fold(W_gate)
    Wcl, Wcli = fold(W_cand)
    # state stationaries: gate [128, (m*2+h)*128], cand [128, m*128]
    wgs = np.zeros((128, 10 * 128), np.float32)
    for m in range(5):
        bd = blockdiag2(Wgl[m][D_IN:])                    # [128, 256]
        for h in range(2):
            # po = (b', oo) with oo = 64h..64h+63
            blk = np.zeros((128, 128), np.float32)
            blk[:64, :64] = Wgl[m][D_IN:, 64 * h:64 * h + 64]
            blk[64:, 64:] = Wgl[m][D_IN:, 64 * h:64 * h + 64]
            wgs[:, (2 * m + h) * 128:(2 * m + h + 1) * 128] = blk
    wcs = np.zeros((128, 5 * 128), np.float32)
    for m in range(5):
        wcs[:, m * 128:(m + 1) * 128] = blockdiag2(Wcl[m][D_IN:])
    # input stationaries: rows m*16 + b*2 + fi, unscaled (xin exact on host)
    wig = np.zeros((5 * PKM, 8 * 128), np.float32)
    wic = np.zeros((5 * PKM, 4 * 128), np.float32)
    for j in range(J):
        for bb in range(2):
            b = 2 * j + bb
            for m in range(5):
                rows = slice(m * PKM + b * 2, m * PKM + b * 2 + 2)
                for h in range(2):
                    wig[rows, (2 * j + h) * 128 + bb * 64:
                        (2 * j + h) * 128 + bb * 64 + 64] = \
                        Wgli[m][:, 64 * h:64 * h + 64]
                wic[rows, j * 128 + bb * 64:j * 128 + bb * 64 + 64] = \
                    Wcli[m][:, :]
    bgh = np.stack([np.tile(np.asarray(b_gate, np.float32)[:64], 2),
                    np.tile(np.asarray(b_gate, np.float32)[64:], 2)], 1)
    bcv = np.tile(np.asarray(b_cand, np.float32), 2).reshape(128, 1)

    in_maps = []
    for cc in range(NCORES):
        bsl = slice(cc * BL, (cc + 1) * BL)
        st_c = state[bsl].reshape(BL, N, U)
        in_c = inputs[bsl].reshape(BL, N, D_IN)
        x0 = np.empty((N, CW), np.float32)
        x0[:, :SC] = st_c.transpose(1, 0, 2).reshape(N, SC)
        x0[:, SC:] = in_c.transpose(1, 0, 2).reshape(N, IC)
        stT = st_c.reshape(J, 2, N, U).transpose(0, 1, 3, 2) \
            .reshape(J, 128, N).transpose(1, 0, 2).reshape(128, J * N)
        # host input diffusion: xin_m = S_m @ x_in (exact fp32)
        xin_nat = in_c.transpose(1, 0, 2).reshape(N, IC)   # [n, (b, fi)]
        xin = np.empty((5 * PKM, N), np.float32)
        xin[:PKM] = xin_nat.T
        for m in range(1, 5):
            xin[m * PKM:(m + 1) * PKM] = (smatsT[m - 1].T @ xin_nat).T
        m = dict(x0=x0.astype(F8),
                 stT=stT.astype(BF), xin=xin.astype(BF),
                 wgs=wgs.astype(BF), wcs=wcs.astype(BF),
                 wig=wig.astype(BF), wic=wic.astype(BF),
                 bg=bgh, bc=bcv)
        for i, sm in enumerate(smats):
            m[f"s{i + 1}"] = sm
        in_maps.append(m)
    return in_maps


def _assemble(res):
    outs = []
    for cc in range(NCORES):
        o = np.asarray(res.results[cc]["out"]).astype(np.float32)  # [128, J*N]
        o = o.reshape(2, U, J, N)            # [b', f, j, n]
        o = o.transpose(2, 0, 3, 1)          # [j, b', n, f]
        outs.append(o.reshape(BL, N * U))
    return np.concatenate(outs, 0)


def kernel(**inputs):
    nc = _get_nc()
    in_maps = _host_prep(**inputs)
    res = run_bass_kernel_spmd(nc, in_maps, list(range(NCORES)))
    return _assemble(res)


# revision 30
# speedup vs baseline: 1.0156x; 1.0156x over previous
"""DCGRU cell on 8 Trainium2 NeuronCores (data-parallel over batch).

Design (v1, feature-major + fp8 DoubleRow):
  - All diffusion terms are direct functions of x0: with the Chebyshev
    fold x2 = 2*S^2 x0 - x0, host precomputes S^T and (S^2)^T per
    support and folds the constants into the projection weights
    (W0' = W0 - W2 - W4, W2' = 2 W2, W4' = 2 W4).  No chained spmm.
  - spmm runs feature-major: stationary = x0 natural node-major blocks,
    moving = S^T column chunks.  Output y^T = (S x)^T lands directly in
    the (batch,feature)-partition layout the projection consumes, so no
    PE transposes of diffusion outputs are needed.
  - Diffusion matmuls are fp8e4m3 with DoubleRow perf mode (256-node
    contraction per instruction, 0.5 cycles/row).  Each S matrix is
    pre-scaled by a power of two into fp8's normal range (S^2 is
    otherwise entirely subnormal); the inverse scale is folded into the
    bf16 projection weights.  Simulated end-to-end rel err: 3.4e-3.
  - Projection stays bf16: stationaries are 2-batch block-diagonal W
    tiles; the (m, input-feature) terms contract via an 80-partition
    packed xin^T tile shared by both dconvs.
  - Gate outputs stay feature-major: u^T kept in SBUF, candidate input
    candX^T = sigmoid(r)^T * state^T built feature-major; 64 small
    transposes produce the fp8 node-major candX stationary.  The final
    GRU combine runs feature-major and the host un-transposes.
"""

import numpy as np

import concourse.bass as bass
from concourse import bacc
import concourse.mybir as mybir
import concourse.tile as tile
from concourse.bass_utils import run_bass_kernel_spmd
from concourse.masks import make_identity

N = 2048            # nodes
B = 64              # global batch
BL = 8              # batch per core
NCORES = 8
D_IN = 2
U = 64              # hidden units
M = 5               # 1 + 2 supports * 2 steps
F = D_IN + U        # 66
NB = N // 128       # 16 node blocks
SC = BL * U         # 512 state cols in natural layout
IC = BL * D_IN      # 16 input cols
CW = SC + IC        # 528 natural cols per node block
NCH = 512           # node chunk (psum free size)
NC4 = N // NCH      # 4 chunks
J = BL // 2         # 4 batch pairs
PKM = 16            # packed input rows per m (8 b * 2 fi)

F32 = mybir.dt.float32
BF16 = mybir.dt.bfloat16
FP8 = mybir.dt.float8e4
DR = mybir.MatmulPerfMode.DoubleRow


def _build_nc():
    nc = bacc.Bacc(None, target_bir_lowering=False)

    x0d = nc.declare_dram_parameter("x0", [N, CW], FP8, isOutput=False)
    stTd = nc.declare_dram_parameter("stT", [128, J * N], BF16, isOutput=False)
    xind = nc.declare_dram_parameter("xin", [5 * PKM, N], BF16, isOutput=False)
    sd = [nc.declare_dram_parameter(f"s{m}", [N, N], FP8, isOutput=False)
          for m in range(1, 5)]
    wgsd = nc.declare_dram_parameter("wgs", [128, 10 * 128], BF16, isOutput=False)
    wcsd = nc.declare_dram_parameter("wcs", [128, 5 * 128], BF16, isOutput=False)
    wigd = nc.declare_dram_parameter("wig", [5 * PKM, 8 * 128], BF16, isOutput=False)
    wicd = nc.declare_dram_parameter("wic", [5 * PKM, 4 * 128], BF16, isOutput=False)
    bgd = nc.declare_dram_parameter("bg", [128, 2], F32, isOutput=False)
    bcd = nc.declare_dram_parameter("bc", [128, 1], F32, isOutput=False)
    outd = nc.declare_dram_parameter("out", [128, J * N], BF16, isOutput=True)

    with tile.TileContext(nc) as tc:
        _emit(nc, tc, x0d, stTd, xind, sd, wgsd, wcsd, wigd, wicd,
              bgd, bcd, outd)
    nc.compile()
    return nc


def _emit(nc, tc, x0d, stTd, xind, sd, wgsd, wcsd, wigd, wicd,
          bgd, bcd, outd):
    from contextlib import ExitStack
    ctx = ExitStack()
    with ctx:
        consts = ctx.enter_context(tc.tile_pool(name="consts", bufs=1))
        acts = ctx.enter_context(tc.tile_pool(name="acts", bufs=1))
        spool = ctx.enter_context(tc.tile_pool(name="spool", bufs=2))
        small = ctx.enter_context(tc.tile_pool(name="small", bufs=3))
        psum = ctx.enter_context(tc.tile_pool(name="psum", bufs=8, space="PSUM"))

        # x0 load goes first so diffusion can start immediately; everything
        # else trails it in the DMA queues.
        x0n = acts.tile([128, NB * CW], FP8, tag="x0n")
        x0dv = x0d.rearrange("(t p) c -> p t c", p=128)
        nc.sync.dma_start(x0n[:].rearrange("p (t c) -> p t c", c=CW), x0dv)

        ident = consts.tile([128, 128], F32)
        make_identity(nc, ident[:])
        identb = consts.tile([128, 128], BF16)
        nc.vector.tensor_copy(identb[:], ident[:])

        wgs = consts.tile([128, 10 * 128], BF16)
        wcs = consts.tile([128, 5 * 128], BF16)
        wig = consts.tile([5 * PKM, 8 * 128], BF16)
        wic = consts.tile([5 * PKM, 4 * 128], BF16)
        bg = consts.tile([128, 2], F32)
        bc = consts.tile([128, 1], F32)

        def load_gate_consts():
            for dst, sr in ((wgs, wgsd), (wig, wigd), (bg, bgd)):
                nc.sync.dma_start(dst[:], sr[:])

        def load_cand_consts():
            for dst, sr in ((wcs, wcsd), (wic, wicd), (bc, bcd)):
                nc.sync.dma_start(dst[:], sr[:])

        # activations
        cxn = acts.tile([128, NB * CW], FP8, tag="cxn")      # natural candX
        stT = acts.tile([128, J * N], BF16, tag="stT")       # state^T
        uT = acts.tile([128, J * N], BF16, tag="uT")
        cxT = acts.tile([128, J * N], BF16, tag="cxT")       # candX^T (state)
        # packed xin^T [(m, b, fi), n], all five m host-precomputed
        xinT = acts.tile([5 * PKM, N], BF16, tag="xinT")
        xsT = acts.tile([128, 16 * NCH], BF16, tag="xsT")    # (m-1, j) chunk slices
        # resident S^T tiles for m=1 (S_a) and m=3 (S_b): loaded chunkwise
        # during the gate phase, reused without DMA in the candidate phase
        sres = {1: acts.tile([128, NB * N], FP8, tag="s1r", name="s1r"),
                3: acts.tile([128, NB * N], FP8, tag="s3r", name="s3r")}


        sdv = [s.rearrange("(jb p) n -> p jb n", p=128) for s in sd]

        def xsT_s(m, j):
            return xsT[:, ((m - 1) * J + j) * NCH:((m - 1) * J + j + 1) * NCH]

        def s_view(c, m, load):
            """Access view of S_m^T chunk c; DMA it if needed."""
            if m in sres:
                scv = sres[m][:].rearrange(
                    "p (jb n) -> p jb n", n=N)[:, :, c * NCH:(c + 1) * NCH]
                if load:
                    nc.sync.dma_start(
                        scv, sdv[m - 1][:, :, c * NCH:(c + 1) * NCH])
            else:
                sc = spool.tile([128, NB * NCH], FP8, tag="sc",
                                name=f"sc{m}")
                scv = sc[:].rearrange("p (jb n) -> p jb n", n=NCH)
                nc.sync.dma_start(
                    scv, sdv[m - 1][:, :, c * NCH:(c + 1) * NCH])
            return scv

        def diffuse_g(scvs, src, g):
            """One batch-pair group g of chunk work: all four S mats."""
            srcv = src[:].rearrange("p (t w) -> p t w", w=CW)
            c0, c1 = g * 128, (g + 1) * 128
            for m in range(1, 5):
                pt = psum.tile([128, NCH], F32, tag="ps", name=f"pd{m}")
                for t in range(8):
                    nc.tensor.matmul(
                        pt[:],
                        srcv[:, 2 * t:2 * t + 2, c0:c1],
                        scvs[m][:, 2 * t:2 * t + 2, :],
                        start=(t == 0), stop=(t == 7), perf_mode=DR)
                nc.vector.tensor_copy(xsT_s(m, g)[:], pt[:])

        def gate_proj(c, j):
                stTs = stT[:, j * N + c * NCH:j * N + (c + 1) * NCH]
                for h in range(2):
                    pp = psum.tile([128, NCH], F32, tag="ps", name="pproj")
                    nc.tensor.matmul(pp[:], wgs[:, h * 128:(h + 1) * 128],
                                     stTs, start=True, stop=False)
                    for m in range(1, 5):
                        nc.tensor.matmul(
                            pp[:], wgs[:, (2 * m + h) * 128:(2 * m + h + 1) * 128],
                            xsT_s(m, j), start=False, stop=False)
                    nc.tensor.matmul(
                        pp[:], wig[:, (2 * j + h) * 128:(2 * j + h + 1) * 128],
                        xinT[:, c * NCH:(c + 1) * NCH],
                        start=False, stop=True)
                    if h == 0:
                        rT = small.tile([128, NCH], BF16, tag="rT")
                        nc.scalar.activation(
                            rT[:], pp[:],
                            mybir.ActivationFunctionType.Sigmoid,
                            bias=bg[:, 0:1])
                        nc.vector.tensor_mul(
                            cxT[:, j * N + c * NCH:j * N + (c + 1) * NCH],
                            rT[:], stTs)
                    else:
                        nc.scalar.activation(
                            uT[:, j * N + c * NCH:j * N + (c + 1) * NCH], pp[:],
                            mybir.ActivationFunctionType.Sigmoid,
                            bias=bg[:, 1:2])

        def candx_nat(c, j):
                tp = psum.tile([128, NCH], BF16, tag="ps", name="ptr")
                for nb in range(4):
                    nc.tensor.transpose(
                        tp[:, nb * 128:(nb + 1) * 128],
                        cxT[:, j * N + c * NCH + nb * 128:
                            j * N + c * NCH + (nb + 1) * 128],
                        identb[:])
                for nb in range(4):
                    i = c * 4 + nb
                    nc.vector.tensor_copy(
                        cxn[:, i * CW + j * 128:i * CW + (j + 1) * 128],
                        tp[:, nb * 128:(nb + 1) * 128])

        def cand_proj(c, j):
                stTs = stT[:, j * N + c * NCH:j * N + (c + 1) * NCH]
                pp = psum.tile([128, NCH], F32, tag="ps", name="pproj")
                nc.tensor.matmul(pp[:], wcs[:, 0:128],
                                 cxT[:, j * N + c * NCH:j * N + (c + 1) * NCH],
                                 start=True, stop=False)
                for m in range(1, 5):
                    nc.tensor.matmul(pp[:], wcs[:, m * 128:(m + 1) * 128],
                                     xsT_s(m, j), start=False, stop=False)
                nc.tensor.matmul(pp[:], wic[:, j * 128:(j + 1) * 128],
                                 xinT[:, c * NCH:(c + 1) * NCH],
                                 start=False, stop=True)
                cT = small.tile([128, NCH], BF16, tag="cT")
                nc.scalar.activation(cT[:], pp[:],
                                     mybir.ActivationFunctionType.Tanh,
                                     bias=bc[:])
                ot = small.tile([128, NCH], BF16, tag="ot")
                uTs = uT[:, j * N + c * NCH:j * N + (c + 1) * NCH]
                nc.gpsimd.tensor_sub(ot[:], stTs, cT[:])
                nc.gpsimd.tensor_mul(ot[:], ot[:], uTs)
                nc.gpsimd.tensor_add(ot[:], ot[:], cT[:])
                nc.sync.dma_start(
                    outd[:, j * N + c * NCH:j * N + (c + 1) * NCH], ot[:])

        def diffuse_m(scvs, src, m, tail=None):
            """All four batch-pair groups for S-matrix m (m-major).
            tail(g), if given, emits the group's projection right after
            its diffusion so the chunk's epilogue pipelines."""
            srcv = src[:].rearrange("p (t w) -> p t w", w=CW)
            for g in range(4):
                c0, c1 = g * 128, (g + 1) * 128
                pt = psum.tile([128, NCH], F32, tag="ps", name=f"pd{g}")
                for t in range(8):
                    nc.tensor.matmul(
                        pt[:],
                        srcv[:, 2 * t:2 * t + 2, c0:c1],
                        scvs[m][:, 2 * t:2 * t + 2, :],
                        start=(t == 0), stop=(t == 7), perf_mode=DR)
                nc.vector.tensor_copy(xsT_s(m, g)[:], pt[:])
                if tail is not None:
                    tail(g)

        # ---- gate ----
        for c in range(NC4):
            scvs = {1: s_view(c, 1, True), 2: s_view(c, 2, True)}
            if c == 0:
                nc.sync.dma_start(stT[:], stTd[:])
            scvs[3] = s_view(c, 3, True)
            scvs[4] = s_view(c, 4, True)
            if c == 0:
                load_gate_consts()
                nc.sync.dma_start(xinT[:], xind[:])
            for m in range(1, 4):
                diffuse_m(scvs, x0n, m)
            diffuse_m(scvs, x0n, 4,
                      tail=lambda g: (gate_proj(c, g), candx_nat(c, g)))
            if c == 0:
                load_cand_consts()
        # ---- candidate ----
        for c in range(NC4):
            scvs = {m: s_view(c, m, False) for m in range(1, 5)}
            for m in range(1, 4):
                diffuse_m(scvs, cxn, m)
            diffuse_m(scvs, cxn, 4, tail=lambda g: cand_proj(c, g))


_NC_CACHE = {}


def _get_nc():
    if "nc" not in _NC_CACHE:
        _NC_CACHE["nc"] = _build_nc()
    return _NC_CACHE["nc"]


def _host_prep(inputs, state, edges1, vals1, edges2, vals2, W_gate, b_gate,
               W_cand, b_cand):
    import ml_dtypes
    BF = ml_dtypes.bfloat16
    # values kept <= 224 so encodings are identical under e4m3 and e4m3fn
    F8 = ml_dtypes.float8_e4m3
    inputs = np.asarray(inputs, np.float32)
    state = np.asarray(state, np.float32)

    def densify_T(edges, vals):
        ST = np.zeros((N, N), np.float32)
        np.add.at(ST, (np.asarray(edges[1]).astype(np.int64),
                       np.asarray(edges[0]).astype(np.int64)),
                  np.asarray(vals, np.float32))
        return ST

    SaT = densify_T(edges1, vals1)
    SbT = densify_T(edges2, vals2)
    Sa2T = SaT @ SaT
    Sb2T = SbT @ SbT
    smats, sscale = [], []
    smatsT = [SaT, Sa2T, SbT, Sb2T]
    for S in smatsT:
        s = 2.0 ** np.floor(np.log2(224.0 / np.abs(S).max()))
        smats.append((S * s).astype(F8))
        sscale.append(s)

    def fold(Wmat):
        Wm = np.asarray(Wmat, np.float32).reshape(F, M, -1).copy()
        Wl = [Wm[:, 0] - Wm[:, 2] - Wm[:, 4], Wm[:, 1], 2.0 * Wm[:, 2],
              Wm[:, 3], 2.0 * Wm[:, 4]]
        Wli = [w[:D_IN].copy() for w in Wl]      # input rows, unscaled
        for m in range(1, 5):
            Wl[m] = Wl[m] / sscale[m - 1]        # state rows absorb 1/s_m
        return Wl, Wli

    def blockdiag2(Wst):
        O = Wst.shape[1]
        Z = np.zeros((128, 2 * O), np.float32)
        Z[:64, :O] = Wst
        Z[64:, O:] = Wst
        return Z

    Wgl, Wgli = fold(W_gate)
    Wcl, Wcli = fold(W_cand)
    # state stationaries: gate [128, (m*2+h)*128], cand [128, m*128]
    wgs = np.zeros((128, 10 * 128), np.float32)
    for m in range(5):
        bd = blockdiag2(Wgl[m][D_IN:])                    # [128, 256]
        for h in range(2):
            # po = (b', oo) with oo = 64h..64h+63
            blk = np.zeros((128, 128), np.float32)
            blk[:64, :64] = Wgl[m][D_IN:, 64 * h:64 * h + 64]
            blk[64:, 64:] = Wgl[m][D_IN:, 64 * h:64 * h + 64]
            wgs[:, (2 * m + h) * 128:(2 * m + h + 1) * 128] = blk
    wcs = np.zeros((128, 5 * 128), np.float32)
    for m in range(5):
        wcs[:, m * 128:(m + 1) * 128] = blockdiag2(Wcl[m][D_IN:])
    # input stationaries: rows m*16 + b*2 + fi, unscaled (xin exact on host)
    wig = np.zeros((5 * PKM, 8 * 128), np.float32)
    wic = np.zeros((5 * PKM, 4 * 128), np.float32)
    for j in range(J):
        for bb in range(2):
            b = 2 * j + bb
            for m in range(5):
                rows = slice(m * PKM + b * 2, m * PKM + b * 2 + 2)
                for h in range(2):
                    wig[rows, (2 * j + h) * 128 + bb * 64:
                        (2 * j + h) * 128 + bb * 64 + 64] = \
                        Wgli[m][:, 64 * h:64 * h + 64]
                wic[rows, j * 128 + bb * 64:j * 128 + bb * 64 + 64] = \
                    Wcli[m][:, :]
    bgh = np.stack([np.tile(np.asarray(b_gate, np.float32)[:64], 2),
                    np.tile(np.asarray(b_gate, np.float32)[64:], 2)], 1)
    bcv = np.tile(np.asarray(b_cand, np.float32), 2).reshape(128, 1)

    in_maps = []
    for cc in range(NCORES):
        bsl = slice(cc * BL, (cc + 1) * BL)
        st_c = state[bsl].reshape(BL, N, U)
        in_c = inputs[bsl].reshape(BL, N, D_IN)
        x0 = np.empty((N, CW), np.float32)
        x0[:, :SC] = st_c.transpose(1, 0, 2).reshape(N, SC)
        x0[:, SC:] = in_c.transpose(1, 0, 2).reshape(N, IC)
        stT = st_c.reshape(J, 2, N, U).transpose(0, 1, 3, 2) \
            .reshape(J, 128, N).transpose(1, 0, 2).reshape(128, J * N)
        # host input diffusion: xin_m = S_m @ x_in (exact fp32)
        xin_nat = in_c.transpose(1, 0, 2).reshape(N, IC)   # [n, (b, fi)]
        xin = np.empty((5 * PKM, N), np.float32)
        xin[:PKM] = xin_nat.T
        for m in range(1, 5):
            xin[m * PKM:(m + 1) * PKM] = (smatsT[m - 1].T @ xin_nat).T
        m = dict(x0=x0.astype(F8),
                 stT=stT.astype(BF), xin=xin.astype(BF),
                 wgs=wgs.astype(BF), wcs=wcs.astype(BF),
                 wig=wig.astype(BF), wic=wic.astype(BF),
                 bg=bgh, bc=bcv)
        for i, sm in enumerate(smats):
            m[f"s{i + 1}"] = sm
        in_maps.append(m)
    return in_maps


def _assemble(res):
    outs = []
    for cc in range(NCORES):
        o = np.asarray(res.results[cc]["out"]).astype(np.float32)  # [128, J*N]
        o = o.reshape(2, U, J, N)            # [b', f, j, n]
        o = o.transpose(2, 0, 3, 1)          # [j, b', n, f]
        outs.append(o.reshape(BL, N * U))
    return np.concatenate(outs, 0)


def kernel(**inputs):
    nc = _get_nc()
    in_maps = _host_prep(**inputs)
    res = run_bass_kernel_spmd(nc, in_maps, list(range(NCORES)))
    return _assemble(res)


# revision 32
# speedup vs baseline: 1.0355x; 1.0196x over previous
"""DCGRU cell on 8 Trainium2 NeuronCores (data-parallel over batch).

Design (v1, feature-major + fp8 DoubleRow):
  - All diffusion terms are direct functions of x0: with the Chebyshev
    fold x2 = 2*S^2 x0 - x0, host precomputes S^T and (S^2)^T per
    support and folds the constants into the projection weights
    (W0' = W0 - W2 - W4, W2' = 2 W2, W4' = 2 W4).  No chained spmm.
  - spmm runs feature-major: stationary = x0 natural node-major blocks,
    moving = S^T column chunks.  Output y^T = (S x)^T lands directly in
    the (batch,feature)-partition layout the projection consumes, so no
    PE transposes of diffusion outputs are needed.
  - Diffusion matmuls are fp8e4m3 with DoubleRow perf mode (256-node
    contraction per instruction, 0.5 cycles/row).  Each S matrix is
    pre-scaled by a power of two into fp8's normal range (S^2 is
    otherwise entirely subnormal); the inverse scale is folded into the
    bf16 projection weights.  Simulated end-to-end rel err: 3.4e-3.
  - Projection stays bf16: stationaries are 2-batch block-diagonal W
    tiles; the (m, input-feature) terms contract via an 80-partition
    packed xin^T tile shared by both dconvs.
  - Gate outputs stay feature-major: u^T kept in SBUF, candidate input
    candX^T = sigmoid(r)^T * state^T built feature-major; 64 small
    transposes produce the fp8 node-major candX stationary.  The final
    GRU combine runs feature-major and the host un-transposes.
"""

import numpy as np

import concourse.bass as bass
from concourse import bacc
import concourse.mybir as mybir
import concourse.tile as tile
from concourse.bass_utils import run_bass_kernel_spmd
from concourse.masks import make_identity

N = 2048            # nodes
B = 64              # global batch
BL = 8              # batch per core
NCORES = 8
D_IN = 2
U = 64              # hidden units
M = 5               # 1 + 2 supports * 2 steps
F = D_IN + U        # 66
NB = N // 128       # 16 node blocks
SC = BL * U         # 512 state cols in natural layout
IC = BL * D_IN      # 16 input cols
CW = SC + IC        # 528 natural cols per node block
NCH = 512           # node chunk (psum free size)
NC4 = N // NCH      # 4 chunks
J = BL // 2         # 4 batch pairs
PKM = 16            # packed input rows per m (8 b * 2 fi)

F32 = mybir.dt.float32
BF16 = mybir.dt.bfloat16
FP8 = mybir.dt.float8e4
DR = mybir.MatmulPerfMode.DoubleRow


def _build_nc():
    nc = bacc.Bacc(None, target_bir_lowering=False)

    x0d = nc.declare_dram_parameter("x0", [128, NB * CW], FP8, isOutput=False)
    stTd = nc.declare_dram_parameter("stT", [128, J * N], BF16, isOutput=False)
    xind = nc.declare_dram_parameter("xin", [5 * PKM, N], BF16, isOutput=False)
    sd = [nc.declare_dram_parameter(f"s{m}", [128, NC4 * NB * NCH], FP8,
                                    isOutput=False)
          for m in range(1, 5)]
    wgsd = nc.declare_dram_parameter("wgs", [128, 10 * 128], BF16, isOutput=False)
    wcsd = nc.declare_dram_parameter("wcs", [128, 5 * 128], BF16, isOutput=False)
    wigd = nc.declare_dram_parameter("wig", [5 * PKM, 8 * 128], BF16, isOutput=False)
    wicd = nc.declare_dram_parameter("wic", [5 * PKM, 4 * 128], BF16, isOutput=False)
    bgd = nc.declare_dram_parameter("bg", [128, 2], F32, isOutput=False)
    bcd = nc.declare_dram_parameter("bc", [128, 1], F32, isOutput=False)
    outd = nc.declare_dram_parameter("out", [128, J * N], BF16, isOutput=True)

    with tile.TileContext(nc) as tc:
        _emit(nc, tc, x0d, stTd, xind, sd, wgsd, wcsd, wigd, wicd,
              bgd, bcd, outd)
    nc.compile()
    return nc


def _emit(nc, tc, x0d, stTd, xind, sd, wgsd, wcsd, wigd, wicd,
          bgd, bcd, outd):  # noqa: sd = list of tiled S dram params
    from contextlib import ExitStack
    ctx = ExitStack()
    with ctx:
        consts = ctx.enter_context(tc.tile_pool(name="consts", bufs=1))
        acts = ctx.enter_context(tc.tile_pool(name="acts", bufs=1))
        spool = ctx.enter_context(tc.tile_pool(name="spool", bufs=2))
        small = ctx.enter_context(tc.tile_pool(name="small", bufs=3))
        psum = ctx.enter_context(tc.tile_pool(name="psum", bufs=8, space="PSUM"))

        # x0 load goes first so diffusion can start immediately; everything
        # else trails it in the DMA queues.
        x0n = acts.tile([128, NB * CW], FP8, tag="x0n")
        nc.sync.dma_start(x0n[:], x0d[:])

        ident = consts.tile([128, 128], F32)
        make_identity(nc, ident[:])
        identb = consts.tile([128, 128], BF16)
        nc.vector.tensor_copy(identb[:], ident[:])

        wgs = consts.tile([128, 10 * 128], BF16)
        wcs = consts.tile([128, 5 * 128], BF16)
        wig = consts.tile([5 * PKM, 8 * 128], BF16)
        wic = consts.tile([5 * PKM, 4 * 128], BF16)
        bg = consts.tile([128, 2], F32)
        bc = consts.tile([128, 1], F32)

        def load_gate_consts():
            for dst, sr in ((wgs, wgsd), (wig, wigd), (bg, bgd)):
                nc.sync.dma_start(dst[:], sr[:])

        def load_cand_consts():
            for dst, sr in ((wcs, wcsd), (wic, wicd), (bc, bcd)):
                nc.sync.dma_start(dst[:], sr[:])

        # activations
        cxn = acts.tile([128, NB * CW], FP8, tag="cxn")      # natural candX
        stT = acts.tile([128, J * N], BF16, tag="stT")       # state^T
        uT = acts.tile([128, J * N], BF16, tag="uT")
        cxT = acts.tile([128, J * N], BF16, tag="cxT")       # candX^T (state)
        # packed xin^T [(m, b, fi), n], all five m host-precomputed
        xinT = acts.tile([5 * PKM, N], BF16, tag="xinT")
        xsT = acts.tile([128, 16 * NCH], BF16, tag="xsT")    # (m-1, j) chunk slices
        # resident S^T tiles for m=1 (S_a) and m=3 (S_b): loaded chunkwise
        # during the gate phase, reused without DMA in the candidate phase
        sres = {1: acts.tile([128, NC4 * NB * NCH], FP8, tag="s1r", name="s1r"),
                3: acts.tile([128, NC4 * NB * NCH], FP8, tag="s3r", name="s3r")}




        def xsT_s(m, j):
            return xsT[:, ((m - 1) * J + j) * NCH:((m - 1) * J + j + 1) * NCH]

        CB = NB * NCH                      # cols per chunk block (8192)

        def s_view(c, m, load):
            """Access view of S_m^T chunk c; DMA it if needed.  DRAM holds
            the chunk-major tiled layout so loads are partition-contiguous."""
            if m in sres:
                flat = sres[m][:, c * CB:(c + 1) * CB]
                if load:
                    nc.sync.dma_start(flat, sd[m - 1][:, c * CB:(c + 1) * CB])
            else:
                sc = spool.tile([128, CB], FP8, tag="sc", name=f"sc{m}")
                flat = sc[:]
                nc.sync.dma_start(flat, sd[m - 1][:, c * CB:(c + 1) * CB])
            return flat.rearrange("p (jb n) -> p jb n", n=NCH)

        def diffuse_g(scvs, src, g):
            """One batch-pair group g of chunk work: all four S mats."""
            srcv = src[:].rearrange("p (t w) -> p t w", w=CW)
            c0, c1 = g * 128, (g + 1) * 128
            for m in range(1, 5):
                pt = psum.tile([128, NCH], F32, tag="ps", name=f"pd{m}")
                for t in range(8):
                    nc.tensor.matmul(
                        pt[:],
                        srcv[:, 2 * t:2 * t + 2, c0:c1],
                        scvs[m][:, 2 * t:2 * t + 2, :],
                        start=(t == 0), stop=(t == 7), perf_mode=DR)
                nc.vector.tensor_copy(xsT_s(m, g)[:], pt[:])

        def gate_proj(c, j):
                stTs = stT[:, j * N + c * NCH:j * N + (c + 1) * NCH]
                for h in range(2):
                    pp = psum.tile([128, NCH], F32, tag="ps", name="pproj")
                    nc.tensor.matmul(pp[:], wgs[:, h * 128:(h + 1) * 128],
                                     stTs, start=True, stop=False)
                    for m in range(1, 5):
                        nc.tensor.matmul(
                            pp[:], wgs[:, (2 * m + h) * 128:(2 * m + h + 1) * 128],
                            xsT_s(m, j), start=False, stop=False)
                    nc.tensor.matmul(
                        pp[:], wig[:, (2 * j + h) * 128:(2 * j + h + 1) * 128],
                        xinT[:, c * NCH:(c + 1) * NCH],
                        start=False, stop=True)
                    if h == 0:
                        rT = small.tile([128, NCH], BF16, tag="rT")
                        nc.scalar.activation(
                            rT[:], pp[:],
                            mybir.ActivationFunctionType.Sigmoid,
                            bias=bg[:, 0:1])
                        nc.vector.tensor_mul(
                            cxT[:, j * N + c * NCH:j * N + (c + 1) * NCH],
                            rT[:], stTs)
                    else:
                        nc.scalar.activation(
                            uT[:, j * N + c * NCH:j * N + (c + 1) * NCH], pp[:],
                            mybir.ActivationFunctionType.Sigmoid,
                            bias=bg[:, 1:2])

        def candx_nat(c, j):
                tp = psum.tile([128, NCH], BF16, tag="ps", name="ptr")
                for nb in range(4):
                    nc.tensor.transpose(
                        tp[:, nb * 128:(nb + 1) * 128],
                        cxT[:, j * N + c * NCH + nb * 128:
                            j * N + c * NCH + (nb + 1) * 128],
                        identb[:])
                for nb in range(4):
                    i = c * 4 + nb
                    nc.vector.tensor_copy(
                        cxn[:, i * CW + j * 128:i * CW + (j + 1) * 128],
                        tp[:, nb * 128:(nb + 1) * 128])

        def cand_proj(c, j):
                stTs = stT[:, j * N + c * NCH:j * N + (c + 1) * NCH]
                pp = psum.tile([128, NCH], F32, tag="ps", name="pproj")
                nc.tensor.matmul(pp[:], wcs[:, 0:128],
                                 cxT[:, j * N + c * NCH:j * N + (c + 1) * NCH],
                                 start=True, stop=False)
                for m in range(1, 5):
                    nc.tensor.matmul(pp[:], wcs[:, m * 128:(m + 1) * 128],
                                     xsT_s(m, j), start=False, stop=False)
                nc.tensor.matmul(pp[:], wic[:, j * 128:(j + 1) * 128],
                                 xinT[:, c * NCH:(c + 1) * NCH],
                                 start=False, stop=True)
                cT = small.tile([128, NCH], BF16, tag="cT")
                nc.scalar.activation(cT[:], pp[:],
                                     mybir.ActivationFunctionType.Tanh,
                                     bias=bc[:])
                ot = small.tile([128, NCH], BF16, tag="ot")
                uTs = uT[:, j * N + c * NCH:j * N + (c + 1) * NCH]
                nc.vector.tensor_sub(ot[:], stTs, cT[:])
                nc.vector.tensor_mul(ot[:], ot[:], uTs)
                nc.vector.tensor_add(ot[:], ot[:], cT[:])
                nc.sync.dma_start(
                    outd[:, j * N + c * NCH:j * N + (c + 1) * NCH], ot[:])

        def diffuse_m(scvs, src, m, tail=None):
            """All four batch-pair groups for S-matrix m (m-major).
            tail(g), if given, emits the group's projection right after
            its diffusion so the chunk's epilogue pipelines."""
            srcv = src[:].rearrange("p (t w) -> p t w", w=CW)
            for g in range(4):
                c0, c1 = g * 128, (g + 1) * 128
                pt = psum.tile([128, NCH], F32, tag="ps", name=f"pd{g}")
                for t in range(8):
                    nc.tensor.matmul(
                        pt[:],
                        srcv[:, 2 * t:2 * t + 2, c0:c1],
                        scvs[m][:, 2 * t:2 * t + 2, :],
                        start=(t == 0), stop=(t == 7), perf_mode=DR)
                nc.vector.tensor_copy(xsT_s(m, g)[:], pt[:])
                if tail is not None:
                    tail(g)

        # ---- gate ----
        for c in range(NC4):
            scvs = {1: s_view(c, 1, True), 2: s_view(c, 2, True)}
            if c == 0:
                nc.sync.dma_start(stT[:], stTd[:])
            scvs[3] = s_view(c, 3, True)
            scvs[4] = s_view(c, 4, True)
            if c == 0:
                load_gate_consts()
                nc.sync.dma_start(xinT[:], xind[:])
            for m in range(1, 4):
                diffuse_m(scvs, x0n, m)
            diffuse_m(scvs, x0n, 4,
                      tail=lambda g: (gate_proj(c, g), candx_nat(c, g)))
            if c == 0:
                load_cand_consts()
        # ---- candidate ----
        for c in range(NC4):
            scvs = {m: s_view(c, m, False) for m in range(1, 5)}
            for m in range(1, 4):
                diffuse_m(scvs, cxn, m)
            diffuse_m(scvs, cxn, 4, tail=lambda g: cand_proj(c, g))


_NC_CACHE = {}


def _get_nc():
    if "nc" not in _NC_CACHE:
        _NC_CACHE["nc"] = _build_nc()
    return _NC_CACHE["nc"]


def _host_prep(inputs, state, edges1, vals1, edges2, vals2, W_gate, b_gate,
               W_cand, b_cand):
    import ml_dtypes
    BF = ml_dtypes.bfloat16
    # values kept <= 224 so encodings are identical under e4m3 and e4m3fn
    F8 = ml_dtypes.float8_e4m3
    inputs = np.asarray(inputs, np.float32)
    state = np.asarray(state, np.float32)

    def densify_T(edges, vals):
        ST = np.zeros((N, N), np.float32)
        np.add.at(ST, (np.asarray(edges[1]).astype(np.int64),
                       np.asarray(edges[0]).astype(np.int64)),
                  np.asarray(vals, np.float32))
        return ST

    SaT = densify_T(edges1, vals1)
    SbT = densify_T(edges2, vals2)
    Sa2T = SaT @ SaT
    Sb2T = SbT @ SbT
    smats, sscale = [], []
    smatsT = [SaT, Sa2T, SbT, Sb2T]
    for S in smatsT:
        s = 2.0 ** np.floor(np.log2(224.0 / np.abs(S).max()))
        tiled = (S * s).reshape(NB, 128, NC4, NCH).transpose(1, 2, 0, 3) \
            .reshape(128, NC4 * NB * NCH)
        smats.append(tiled.astype(F8))
        sscale.append(s)

    def fold(Wmat):
        Wm = np.asarray(Wmat, np.float32).reshape(F, M, -1).copy()
        Wl = [Wm[:, 0] - Wm[:, 2] - Wm[:, 4], Wm[:, 1], 2.0 * Wm[:, 2],
              Wm[:, 3], 2.0 * Wm[:, 4]]
        Wli = [w[:D_IN].copy() for w in Wl]      # input rows, unscaled
        for m in range(1, 5):
            Wl[m] = Wl[m] / sscale[m - 1]        # state rows absorb 1/s_m
        return Wl, Wli

    def blockdiag2(Wst):
        O = Wst.shape[1]
        Z = np.zeros((128, 2 * O), np.float32)
        Z[:64, :O] = Wst
        Z[64:, O:] = Wst
        return Z

    Wgl, Wgli = fold(W_gate)
    Wcl, Wcli = fold(W_cand)
    # state stationaries: gate [128, (m*2+h)*128], cand [128, m*128]
    wgs = np.zeros((128, 10 * 128), np.float32)
    for m in range(5):
        bd = blockdiag2(Wgl[m][D_IN:])                    # [128, 256]
        for h in range(2):
            # po = (b', oo) with oo = 64h..64h+63
            blk = np.zeros((128, 128), np.float32)
            blk[:64, :64] = Wgl[m][D_IN:, 64 * h:64 * h + 64]
            blk[64:, 64:] = Wgl[m][D_IN:, 64 * h:64 * h + 64]
            wgs[:, (2 * m + h) * 128:(2 * m + h + 1) * 128] = blk
    wcs = np.zeros((128, 5 * 128), np.float32)
    for m in range(5):
        wcs[:, m * 128:(m + 1) * 128] = blockdiag2(Wcl[m][D_IN:])
    # input stationaries: rows m*16 + b*2 + fi, unscaled (xin exact on host)
    wig = np.zeros((5 * PKM, 8 * 128), np.float32)
    wic = np.zeros((5 * PKM, 4 * 128), np.float32)
    for j in range(J):
        for bb in range(2):
            b = 2 * j + bb
            for m in range(5):
                rows = slice(m * PKM + b * 2, m * PKM + b * 2 + 2)
                for h in range(2):
                    wig[rows, (2 * j + h) * 128 + bb * 64:
                        (2 * j + h) * 128 + bb * 64 + 64] = \
                        Wgli[m][:, 64 * h:64 * h + 64]
                wic[rows, j * 128 + bb * 64:j * 128 + bb * 64 + 64] = \
                    Wcli[m][:, :]
    bgh = np.stack([np.tile(np.asarray(b_gate, np.float32)[:64], 2),
                    np.tile(np.asarray(b_gate, np.float32)[64:], 2)], 1)
    bcv = np.tile(np.asarray(b_cand, np.float32), 2).reshape(128, 1)

    in_maps = []
    for cc in range(NCORES):
        bsl = slice(cc * BL, (cc + 1) * BL)
        st_c = state[bsl].reshape(BL, N, U)
        in_c = inputs[bsl].reshape(BL, N, D_IN)
        x0 = np.empty((N, CW), np.float32)
        x0[:, :SC] = st_c.transpose(1, 0, 2).reshape(N, SC)
        x0[:, SC:] = in_c.transpose(1, 0, 2).reshape(N, IC)
        x0 = x0.reshape(NB, 128, CW).transpose(1, 0, 2).reshape(128, NB * CW)
        stT = st_c.reshape(J, 2, N, U).transpose(0, 1, 3, 2) \
            .reshape(J, 128, N).transpose(1, 0, 2).reshape(128, J * N)
        # host input diffusion: xin_m = S_m @ x_in (exact fp32)
        xin_nat = in_c.transpose(1, 0, 2).reshape(N, IC)   # [n, (b, fi)]
        xin = np.empty((5 * PKM, N), np.float32)
        xin[:PKM] = xin_nat.T
        for m in range(1, 5):
            xin[m * PKM:(m + 1) * PKM] = (smatsT[m - 1].T @ xin_nat).T
        m = dict(x0=x0.astype(F8),
                 stT=stT.astype(BF), xin=xin.astype(BF),
                 wgs=wgs.astype(BF), wcs=wcs.astype(BF),
                 wig=wig.astype(BF), wic=wic.astype(BF),
                 bg=bgh, bc=bcv)
        for i, sm in enumerate(smats):
            m[f"s{i + 1}"] = sm
        in_maps.append(m)
    return in_maps


def _assemble(res):
    outs = []
    for cc in range(NCORES):
        o = np.asarray(res.results[cc]["out"]).astype(np.float32)  # [128, J*N]
        o = o.reshape(2, U, J, N)            # [b', f, j, n]
        o = o.transpose(2, 0, 3, 1)          # [j, b', n, f]
        outs.append(o.reshape(BL, N * U))
    return np.concatenate(outs, 0)


def kernel(**inputs):
    nc = _get_nc()
    in_maps = _host_prep(**inputs)
    res = run_bass_kernel_spmd(nc, in_maps, list(range(NCORES)))
    return _assemble(res)
